# revision 11
# baseline (speedup 1.0000x reference)
"""Trainium2 Bass kernel for nn_Attention_63273458205325.

Data-parallel over batch: 64 images -> 8 NeuronCores x 8 images.
Device kernel computes, per image, the four memory-bound global
reductions over x[b] (256x4096 fp32):
  - beta row-sums  (per-channel sum over spatial)       [256]
  - mask logits m = w_mask . x  -> exp -> Z and the
    softmax-weighted context sums  sum_s x[c,s]*e[s]    [256]
  - mean over spatial of (max over channels)            scalar
The tiny [B,8] epilogue head (layernorm/gelu/1x1 convs/sigmoid/softmax
on 256-vectors) runs on host.
"""

import sys

import numpy as np

sys.path.insert(0, "/opt/trn_rl_repo")

B, C, H, W = 64, 256, 64, 64
S = H * W  # 4096
NCORES = 8
BPC = B // NCORES  # images per core
RATIO, K = 16, 8
PLANES = C // 2
HIDDEN = C // RATIO
TEMP = 30.0
EPS = 1e-5

_CACHE = {}


def _build_nc():
    import concourse.bacc as bacc
    import concourse.mybir as mybir
    from concourse.tile import TileContext

    f32 = mybir.dt.float32
    bf16 = mybir.dt.bfloat16
    AF = mybir.ActivationFunctionType
    ALU = mybir.AluOpType
    AX = mybir.AxisListType

    nc = bacc.Bacc(None, target_bir_lowering=False)

    x_ext = nc.declare_dram_parameter("x", [BPC, C, S], f32, isOutput=False)
    wm_ext = nc.declare_dram_parameter("wm", [C], f32, isOutput=False)
    ones_ext = nc.declare_dram_parameter("ones1", [1, 128], bf16, isOutput=False)
    id_ext = nc.declare_dram_parameter("ident", [128, 128], bf16, isOutput=False)
    out_ext = nc.declare_dram_parameter("out", [BPC, 128, 8], f32, isOutput=True)

    with TileContext(nc) as tc:
        with (
            tc.tile_pool(name="const", bufs=1) as cpool,
            tc.tile_pool(name="xin", bufs=2) as xpool,
            tc.tile_pool(name="work", bufs=2) as wpool,
            tc.tile_pool(name="fold", bufs=1) as fpool,
            tc.tile_pool(name="small", bufs=2) as spool,
            tc.tile_pool(name="psum", bufs=1, space="PSUM") as ppool,
        ):
            # constants
            wm = cpool.tile([128, 2], f32)  # wm[p, g] = w_mask[g*128 + p]
            nc.sync.dma_start(out=wm[:], in_=wm_ext.rearrange("(g p) -> p g", p=128))
            ones1 = cpool.tile([1, 128], bf16)
            nc.sync.dma_start(out=ones1[:], in_=ones_ext[:])
            ident = cpool.tile([128, 128], bf16)
            nc.sync.dma_start(out=ident[:], in_=id_ext[:])

            for b in range(BPC):
                # ---- load both channel halves [128, 4096] fp32
                x0 = xpool.tile([128, S], f32, tag="x0")
                nc.sync.dma_start(out=x0[:], in_=x_ext[b, 0:128, :])
                x1 = xpool.tile([128, S], f32, tag="x1")
                nc.sync.dma_start(out=x1[:], in_=x_ext[b, 128:256, :])

                stage = spool.tile([128, 8], f32, tag="stage")

                # ---- cast to bf16 on ACT; channel row-sums ride along for free
                xb0 = wpool.tile([128, S], bf16, tag="xb0")
                nc.scalar.activation(xb0[:], x0[:], AF.Copy,
                                     accum_out=stage[:, 0:1])
                xb1 = wpool.tile([128, S], bf16, tag="xb1")
                nc.scalar.activation(xb1[:], x1[:], AF.Copy,
                                     accum_out=stage[:, 1:2])

                # ---- mask logits m = w . x  -> PSUM, two [1, 2048] halves
                #      e = exp(m) (bf16 row); Z = sum(e) split into 2 accums
                e_row = spool.tile([1, S], bf16, tag="e")
                for h in range(2):
                    m_ps = ppool.tile([1, S // 2], f32, tag="m")
                    for j in range(4):
                        sl = slice(512 * j, 512 * (j + 1))
                        gl = slice(2048 * h + 512 * j, 2048 * h + 512 * (j + 1))
                        nc.tensor.matmul(m_ps[:, sl], lhsT=wm[:, 0:1],
                                         rhs=x0[:, gl], start=True, stop=False)
                        nc.tensor.matmul(m_ps[:, sl], lhsT=wm[:, 1:2],
                                         rhs=x1[:, gl], start=False, stop=True)
                    nc.scalar.activation(e_row[:, 2048 * h:2048 * (h + 1)],
                                         m_ps[:], AF.Exp,
                                         accum_out=stage[0:1, 5 + h:6 + h])

                # ---- broadcast e across partitions via K=1 matmul, 512 at a time
                eb = wpool.tile([128, S], bf16, tag="eb_sb")
                for j in range(8):
                    sl = slice(512 * j, 512 * (j + 1))
                    eb_ps = ppool.tile([128, 512], f32, tag="eb")
                    nc.tensor.matmul(eb_ps[:], lhsT=ones1[:], rhs=e_row[:, sl],
                                     start=True, stop=True)
                    nc.scalar.activation(eb[:, sl], eb_ps[:], AF.Copy)

                # ---- context sums: ctx[c] = sum_s x[c,s] * e[s]  (fused mul+acc)
                scr = fpool.tile([128, S], bf16, tag="scr")
                nc.vector.scalar_tensor_tensor(
                    out=scr[:], in0=xb0[:], scalar=1.0, in1=eb[:],
                    op0=ALU.mult, op1=ALU.mult, accum_out=stage[:, 2:3])
                nc.vector.scalar_tensor_tensor(
                    out=scr[:], in0=xb1[:], scalar=1.0, in1=eb[:],
                    op0=ALU.mult, op1=ALU.mult, accum_out=stage[:, 3:4])

                # ---- channel max: fold 256 -> 128 on gpsimd, then PE-transpose
                #      [128,128] chunks so the remaining 128-way max is a DVE
                #      free-axis reduce
                pm = wpool.tile([128, S], bf16, tag="pm")
                nc.vector.tensor_max(pm[:], xb0[:], xb1[:])
                rm = spool.tile([128, 32], bf16, tag="rm")
                for g in range(4):
                    ct_ps = ppool.tile([128, 1024], bf16, tag="ct")
                    for j in range(8):
                        cj = 8 * g + j
                        nc.tensor.transpose(ct_ps[:, 128 * j:128 * (j + 1)],
                                            pm[:, 128 * cj:128 * (cj + 1)],
                                            ident[:])
                    nc.vector.tensor_reduce(
                        rm[:, 8 * g:8 * (g + 1)],
                        ct_ps[:].rearrange("p (j c) -> p j c", c=128),
                        axis=AX.X, op=ALU.max)
                nc.vector.tensor_reduce(stage[:, 4:5], rm[:], axis=AX.X,
                                        op=ALU.add)

                nc.sync.dma_start(out=out_ext[b], in_=stage[:])
    return nc


def _get_nc():
    if "nc" not in _CACHE:
        nc = _build_nc()
        nc.finalize()
        _CACHE["nc"] = nc
    return _CACHE["nc"]


def _run_device(x_np, trace=False):
    """x_np: [64, 256, 64, 64] fp32 -> list of 8 per-core result dicts."""
    import ml_dtypes
    from concourse.bass_utils import run_bass_kernel_spmd

    nc = _get_nc()
    xs = x_np.reshape(NCORES, BPC, C, S)
    wm = np.zeros([C], dtype=np.float32)
    wm[:] = _CACHE["w_mask"].reshape(C)
    ones1 = np.ones([1, 128], dtype=ml_dtypes.bfloat16)
    ident = np.eye(128, dtype=ml_dtypes.bfloat16)
    in_maps = [
        {"x": np.ascontiguousarray(xs[i]), "wm": wm, "ones1": ones1, "ident": ident}
        for i in range(NCORES)
    ]
    res = run_bass_kernel_spmd(nc, in_maps, core_ids=list(range(NCORES)),
                               trace=trace)
    return res


def kernel(x, w_mask, b_mask, w_cm1, b_cm1, ln_w, ln_b, w_cm2, b_cm2,
           w_net1, w_net2, w_fc, bn_w, bn_b, bn_mean, bn_var, w_kfc):
    x = np.asarray(x, dtype=np.float32)
    _CACHE["w_mask"] = np.asarray(w_mask, dtype=np.float32)
    res = _run_device(x)

    # ---- gather device results
    beta_sums = np.zeros([B, C], np.float32)
    ctx_sums = np.zeros([B, C], np.float32)
    zs = np.zeros([B], np.float32)
    cmax_sums = np.zeros([B], np.float32)
    for i in range(NCORES):
        o = np.asarray(res.results[i]["out"], np.float32)  # [BPC, 128, 8]
        for bb in range(BPC):
            g = i * BPC + bb
            beta_sums[g, 0:128] = o[bb, :, 0]
            beta_sums[g, 128:256] = o[bb, :, 1]
            ctx_sums[g, 0:128] = o[bb, :, 2]
            ctx_sums[g, 128:256] = o[bb, :, 3]
            cmax_sums[g] = o[bb, :, 4].sum()
            zs[g] = o[bb, 0, 5] + o[bb, 0, 6]

    # ---- tiny epilogue head on host (mirrors reference.py)
    w_cm1 = np.asarray(w_cm1, np.float32); b_cm1 = np.asarray(b_cm1, np.float32)
    ln_w = np.asarray(ln_w, np.float32); ln_b = np.asarray(ln_b, np.float32)
    w_cm2 = np.asarray(w_cm2, np.float32); b_cm2 = np.asarray(b_cm2, np.float32)
    w_net1 = np.asarray(w_net1, np.float32); w_net2 = np.asarray(w_net2, np.float32)
    w_fc = np.asarray(w_fc, np.float32); bn_w = np.asarray(bn_w, np.float32)
    bn_b = np.asarray(bn_b, np.float32); bn_mean = np.asarray(bn_mean, np.float32)
    bn_var = np.asarray(bn_var, np.float32); w_kfc = np.asarray(w_kfc, np.float32)

    from scipy.special import erf  # exact gelu, matches jax approximate=False

    beta_c = beta_sums / S
    context = ctx_sums / zs[:, None]
    a = beta_sums.sum(axis=1) / (C * S)
    mm = cmax_sums / S
    beta_s = np.zeros([B, C], np.float32)
    beta_s[:, 0::2] = a[:, None]
    beta_s[:, 1::2] = mm[:, None]

    t = context @ w_cm1.T + b_cm1
    mu = t.mean(axis=-1, keepdims=True)
    var = ((t - mu) ** 2).mean(axis=-1, keepdims=True)
    t = (t - mu) / np.sqrt(var + EPS) * ln_w + ln_b
    t = t * 0.5 * (1.0 + erf(t / np.sqrt(2.0)))
    beta_g = t @ w_cm2.T + b_cm2

    out = beta_c + beta_g + beta_s
    out = np.maximum(out @ w_net1.T, 0.0) @ w_net2.T  # [B, K]

    ka = out @ w_fc.T
    ka = (ka - bn_mean) / np.sqrt(bn_var + EPS) * bn_w + bn_b
    kat = 1.0 / (1.0 + np.exp(-(np.maximum(ka, 0.0) @ w_kfc.T)))
    out = out * kat
    out = out / TEMP
    out = out - out.max(axis=-1, keepdims=True)
    e = np.exp(out)
    return (e / e.sum(axis=-1, keepdims=True)).astype(np.float32)


# revision 13
# speedup vs baseline: 385.6237x; 385.6237x over previous
"""Trainium2 Bass kernel for nn_Attention_63273458205325.

Data-parallel over batch: 64 images -> 8 NeuronCores x 8 images.
Device kernel computes, per image, the four memory-bound global
reductions over x[b] (256x4096 fp32):
  - beta row-sums  (per-channel sum over spatial)       [256]
  - mask logits m = w_mask . x  -> exp -> Z and the
    softmax-weighted context sums  sum_s x[c,s]*e[s]    [256]
  - mean over spatial of (max over channels)            scalar
The tiny [B,8] epilogue head (layernorm/gelu/1x1 convs/sigmoid/softmax
on 256-vectors) runs on host.
"""

import sys

import numpy as np

sys.path.insert(0, "/opt/trn_rl_repo")

B, C, H, W = 64, 256, 64, 64
S = H * W  # 4096
NCORES = 8
BPC = B // NCORES  # images per core
RATIO, K = 16, 8
PLANES = C // 2
HIDDEN = C // RATIO
TEMP = 30.0
EPS = 1e-5

_CACHE = {}


def _build_nc():
    import concourse.bacc as bacc
    import concourse.mybir as mybir
    from concourse.tile import TileContext

    f32 = mybir.dt.float32
    bf16 = mybir.dt.bfloat16
    AF = mybir.ActivationFunctionType
    ALU = mybir.AluOpType
    AX = mybir.AxisListType

    nc = bacc.Bacc(None, target_bir_lowering=False)

    x_ext = nc.declare_dram_parameter("x", [BPC, C, S], f32, isOutput=False)
    wm_ext = nc.declare_dram_parameter("wm", [C], f32, isOutput=False)
    ones_ext = nc.declare_dram_parameter("ones1", [1, 128], bf16, isOutput=False)
    id_ext = nc.declare_dram_parameter("ident", [128, 128], bf16, isOutput=False)
    out_ext = nc.declare_dram_parameter("out", [BPC, 128, 8], f32, isOutput=True)

    with TileContext(nc) as tc:
        with (
            tc.tile_pool(name="const", bufs=1) as cpool,
            tc.tile_pool(name="xin", bufs=2) as xpool,
            tc.tile_pool(name="work", bufs=2) as wpool,
            tc.tile_pool(name="fold", bufs=1) as fpool,
            tc.tile_pool(name="small", bufs=2) as spool,
            tc.tile_pool(name="psum", bufs=1, space="PSUM") as ppool,
        ):
            # constants
            wm = cpool.tile([128, 2], f32)  # wm[p, g] = w_mask[g*128 + p]
            nc.sync.dma_start(out=wm[:], in_=wm_ext.rearrange("(g p) -> p g", p=128))
            ones1 = cpool.tile([1, 128], bf16)
            nc.sync.dma_start(out=ones1[:], in_=ones_ext[:])
            ident = cpool.tile([128, 128], bf16)
            nc.sync.dma_start(out=ident[:], in_=id_ext[:])

            for b in range(BPC):
                # ---- load both channel halves [128, 4096] fp32
                x0 = xpool.tile([128, S], f32, tag="x0")
                nc.sync.dma_start(out=x0[:], in_=x_ext[b, 0:128, :])
                x1 = xpool.tile([128, S], f32, tag="x1")
                nc.sync.dma_start(out=x1[:], in_=x_ext[b, 128:256, :])

                stage = spool.tile([128, 8], f32, tag="stage")

                # ---- cast to bf16 on ACT; channel row-sums ride along for free
                xb0 = wpool.tile([128, S], bf16, tag="xb0")
                nc.scalar.activation(xb0[:], x0[:], AF.Copy,
                                     accum_out=stage[:, 0:1])
                xb1 = wpool.tile([128, S], bf16, tag="xb1")
                nc.scalar.activation(xb1[:], x1[:], AF.Copy,
                                     accum_out=stage[:, 1:2])

                # ---- mask logits m = w . x  -> PSUM, two [1, 2048] halves
                #      e = exp(m) (bf16 row); Z = sum(e) split into 2 accums
                e_row = spool.tile([1, S], bf16, tag="e")
                for h in range(2):
                    m_ps = ppool.tile([1, S // 2], f32, tag="m")
                    for j in range(4):
                        sl = slice(512 * j, 512 * (j + 1))
                        gl = slice(2048 * h + 512 * j, 2048 * h + 512 * (j + 1))
                        nc.tensor.matmul(m_ps[:, sl], lhsT=wm[:, 0:1],
                                         rhs=x0[:, gl], start=True, stop=False)
                        nc.tensor.matmul(m_ps[:, sl], lhsT=wm[:, 1:2],
                                         rhs=x1[:, gl], start=False, stop=True)
                    nc.scalar.activation(e_row[:, 2048 * h:2048 * (h + 1)],
                                         m_ps[:], AF.Exp,
                                         accum_out=stage[0:1, 5 + h:6 + h])

                # ---- broadcast e across partitions via K=1 matmul, 512 at a time
                eb = wpool.tile([128, S], bf16, tag="eb_sb")
                for j in range(8):
                    sl = slice(512 * j, 512 * (j + 1))
                    eb_ps = ppool.tile([128, 512], f32, tag="eb")
                    nc.tensor.matmul(eb_ps[:], lhsT=ones1[:], rhs=e_row[:, sl],
                                     start=True, stop=True)
                    nc.scalar.activation(eb[:, sl], eb_ps[:], AF.Copy)

                # ---- context sums: ctx[c] = sum_s x[c,s] * e[s]  (fused mul+acc)
                scr = fpool.tile([128, S], bf16, tag="scr")
                nc.vector.scalar_tensor_tensor(
                    out=scr[:], in0=xb0[:], scalar=1.0, in1=eb[:],
                    op0=ALU.mult, op1=ALU.mult, accum_out=stage[:, 2:3])
                nc.vector.scalar_tensor_tensor(
                    out=scr[:], in0=xb1[:], scalar=1.0, in1=eb[:],
                    op0=ALU.mult, op1=ALU.mult, accum_out=stage[:, 3:4])

                # ---- channel max: fold 256 -> 128 on gpsimd, then PE-transpose
                #      [128,128] chunks so the remaining 128-way max is a DVE
                #      free-axis reduce
                pm = wpool.tile([128, S], bf16, tag="pm")
                nc.vector.tensor_max(pm[:], xb0[:], xb1[:])
                rm = spool.tile([128, 32], bf16, tag="rm")
                for g in range(4):
                    ct_ps = ppool.tile([128, 1024], bf16, tag="ct")
                    for j in range(8):
                        cj = 8 * g + j
                        nc.tensor.transpose(ct_ps[:, 128 * j:128 * (j + 1)],
                                            pm[:, 128 * cj:128 * (cj + 1)],
                                            ident[:])
                    nc.vector.tensor_reduce(
                        rm[:, 8 * g:8 * (g + 1)],
                        ct_ps[:].rearrange("p (j c) -> p j c", c=128),
                        axis=AX.X, op=ALU.max)
                nc.vector.tensor_reduce(stage[:, 4:5], rm[:], axis=AX.X,
                                        op=ALU.add)

                nc.sync.dma_start(out=out_ext[b], in_=stage[:])
    return nc


def _get_nc():
    if "nc" not in _CACHE:
        nc = _build_nc()
        nc.finalize()
        _CACHE["nc"] = nc
    return _CACHE["nc"]


def _run_device(x_np, trace=False, tmpdir=None):
    """x_np: [64, 256, 64, 64] fp32 -> list of 8 per-core result dicts."""
    import ml_dtypes
    from concourse.bass_utils import run_bass_kernel_spmd

    nc = _get_nc()
    xs = x_np.reshape(NCORES, BPC, C, S)
    wm = np.zeros([C], dtype=np.float32)
    wm[:] = _CACHE["w_mask"].reshape(C)
    ones1 = np.ones([1, 128], dtype=ml_dtypes.bfloat16)
    ident = np.eye(128, dtype=ml_dtypes.bfloat16)
    in_maps = [
        {"x": np.ascontiguousarray(xs[i]), "wm": wm, "ones1": ones1, "ident": ident}
        for i in range(NCORES)
    ]
    res = run_bass_kernel_spmd(nc, in_maps, core_ids=list(range(NCORES)),
                               trace=trace, tmpdir=tmpdir)
    return res


def kernel(x, w_mask, b_mask, w_cm1, b_cm1, ln_w, ln_b, w_cm2, b_cm2,
           w_net1, w_net2, w_fc, bn_w, bn_b, bn_mean, bn_var, w_kfc):
    x = np.asarray(x, dtype=np.float32)
    _CACHE["w_mask"] = np.asarray(w_mask, dtype=np.float32)
    res = _run_device(x)

    # ---- gather device results
    beta_sums = np.zeros([B, C], np.float32)
    ctx_sums = np.zeros([B, C], np.float32)
    zs = np.zeros([B], np.float32)
    cmax_sums = np.zeros([B], np.float32)
    for i in range(NCORES):
        o = np.asarray(res.results[i]["out"], np.float32)  # [BPC, 128, 8]
        for bb in range(BPC):
            g = i * BPC + bb
            beta_sums[g, 0:128] = o[bb, :, 0]
            beta_sums[g, 128:256] = o[bb, :, 1]
            ctx_sums[g, 0:128] = o[bb, :, 2]
            ctx_sums[g, 128:256] = o[bb, :, 3]
            cmax_sums[g] = o[bb, :, 4].sum()
            zs[g] = o[bb, 0, 5] + o[bb, 0, 6]

    # ---- tiny epilogue head on host (mirrors reference.py)
    w_cm1 = np.asarray(w_cm1, np.float32); b_cm1 = np.asarray(b_cm1, np.float32)
    ln_w = np.asarray(ln_w, np.float32); ln_b = np.asarray(ln_b, np.float32)
    w_cm2 = np.asarray(w_cm2, np.float32); b_cm2 = np.asarray(b_cm2, np.float32)
    w_net1 = np.asarray(w_net1, np.float32); w_net2 = np.asarray(w_net2, np.float32)
    w_fc = np.asarray(w_fc, np.float32); bn_w = np.asarray(bn_w, np.float32)
    bn_b = np.asarray(bn_b, np.float32); bn_mean = np.asarray(bn_mean, np.float32)
    bn_var = np.asarray(bn_var, np.float32); w_kfc = np.asarray(w_kfc, np.float32)

    from scipy.special import erf  # exact gelu, matches jax approximate=False

    beta_c = beta_sums / S
    context = ctx_sums / zs[:, None]
    a = beta_sums.sum(axis=1) / (C * S)
    mm = cmax_sums / S
    beta_s = np.zeros([B, C], np.float32)
    beta_s[:, 0::2] = a[:, None]
    beta_s[:, 1::2] = mm[:, None]

    t = context @ w_cm1.T + b_cm1
    mu = t.mean(axis=-1, keepdims=True)
    var = ((t - mu) ** 2).mean(axis=-1, keepdims=True)
    t = (t - mu) / np.sqrt(var + EPS) * ln_w + ln_b
    t = t * 0.5 * (1.0 + erf(t / np.sqrt(2.0)))
    beta_g = t @ w_cm2.T + b_cm2

    out = beta_c + beta_g + beta_s
    out = np.maximum(out @ w_net1.T, 0.0) @ w_net2.T  # [B, K]

    ka = out @ w_fc.T
    ka = (ka - bn_mean) / np.sqrt(bn_var + EPS) * bn_w + bn_b
    kat = 1.0 / (1.0 + np.exp(-(np.maximum(ka, 0.0) @ w_kfc.T)))
    out = out * kat
    out = out / TEMP
    out = out - out.max(axis=-1, keepdims=True)
    e = np.exp(out)
    return (e / e.sum(axis=-1, keepdims=True)).astype(np.float32)


# revision 16
# speedup vs baseline: 422.8364x; 1.0965x over previous
"""Trainium2 Bass kernel for nn_Attention_63273458205325.

Data-parallel over batch: 64 images -> 8 NeuronCores x 8 images.
Device kernel computes, per image, the four memory-bound global
reductions over x[b] (256x4096 fp32):
  - beta row-sums  (per-channel sum over spatial)       [256]
  - mask logits m = w_mask . x  -> exp -> Z and the
    softmax-weighted context sums  sum_s x[c,s]*e[s]    [256]
  - mean over spatial of (max over channels)            scalar
The tiny [B,8] epilogue head (layernorm/gelu/1x1 convs/sigmoid/softmax
on 256-vectors) runs on host.
"""

import sys

import numpy as np

sys.path.insert(0, "/opt/trn_rl_repo")

B, C, H, W = 64, 256, 64, 64
S = H * W  # 4096
NCORES = 8
BPC = B // NCORES  # images per core
RATIO, K = 16, 8
PLANES = C // 2
HIDDEN = C // RATIO
TEMP = 30.0
EPS = 1e-5

_CACHE = {}


def _build_nc():
    import concourse.bacc as bacc
    import concourse.mybir as mybir
    from concourse.tile import TileContext

    f32 = mybir.dt.float32
    bf16 = mybir.dt.bfloat16
    AF = mybir.ActivationFunctionType
    ALU = mybir.AluOpType
    AX = mybir.AxisListType

    nc = bacc.Bacc(None, target_bir_lowering=False)

    x_ext = nc.declare_dram_parameter("x", [BPC, C, S], f32, isOutput=False)
    wm_ext = nc.declare_dram_parameter("wm", [C], bf16, isOutput=False)
    ones_ext = nc.declare_dram_parameter("ones1", [1, 128], bf16, isOutput=False)
    id_ext = nc.declare_dram_parameter("ident", [128, 128], bf16, isOutput=False)
    out_ext = nc.declare_dram_parameter("out", [BPC, 128, 8], f32, isOutput=True)

    with TileContext(nc) as tc:
        with (
            tc.tile_pool(name="const", bufs=1) as cpool,
            tc.tile_pool(name="xin", bufs=2) as xpool,
            tc.tile_pool(name="work", bufs=2) as wpool,
            tc.tile_pool(name="fold", bufs=1) as fpool,
            tc.tile_pool(name="small", bufs=2) as spool,
            tc.tile_pool(name="psum", bufs=1, space="PSUM") as ppool,
        ):
            # constants
            wm = cpool.tile([128, 2], bf16)  # wm[p, g] = w_mask[g*128 + p]
            nc.sync.dma_start(out=wm[:], in_=wm_ext.rearrange("(g p) -> p g", p=128))
            ones1 = cpool.tile([1, 128], bf16)
            nc.sync.dma_start(out=ones1[:], in_=ones_ext[:])
            ident = cpool.tile([128, 128], bf16)
            nc.sync.dma_start(out=ident[:], in_=id_ext[:])

            for b in range(BPC):
                # ---- load both channel halves [128, 4096] fp32
                x0 = xpool.tile([128, S], f32, tag="x0")
                nc.sync.dma_start(out=x0[:], in_=x_ext[b, 0:128, :])
                x1 = xpool.tile([128, S], f32, tag="x1")
                nc.sync.dma_start(out=x1[:], in_=x_ext[b, 128:256, :])

                stage = spool.tile([128, 8], f32, tag="stage")
                nc.gpsimd.memset(stage[:], 0.0)

                # ---- cast to bf16 on ACT; channel row-sums ride along for free
                xb0 = wpool.tile([128, S], bf16, tag="xb0")
                nc.scalar.activation(xb0[:], x0[:], AF.Copy,
                                     accum_out=stage[:, 0:1])
                xb1 = wpool.tile([128, S], bf16, tag="xb1")
                nc.scalar.activation(xb1[:], x1[:], AF.Copy,
                                     accum_out=stage[:, 1:2])

                # ---- mask logits m = w . x  -> PSUM, two [1, 2048] halves
                #      e = exp(m) (bf16 row); Z = sum(e) split into 2 accums
                e_row = spool.tile([1, S], bf16, tag="e")
                for h in range(2):
                    m_ps = ppool.tile([1, S // 2], f32, tag="m")
                    for j in range(4):
                        sl = slice(512 * j, 512 * (j + 1))
                        gl = slice(2048 * h + 512 * j, 2048 * h + 512 * (j + 1))
                        nc.tensor.matmul(m_ps[:, sl], lhsT=wm[:, 0:1],
                                         rhs=xb0[:, gl], start=True, stop=False)
                        nc.tensor.matmul(m_ps[:, sl], lhsT=wm[:, 1:2],
                                         rhs=xb1[:, gl], start=False, stop=True)
                    nc.scalar.activation(e_row[:, 2048 * h:2048 * (h + 1)],
                                         m_ps[:], AF.Exp,
                                         accum_out=stage[0:1, 5 + h:6 + h])

                # ---- broadcast e across partitions via K=1 matmul, 512 at a time
                eb = wpool.tile([128, S], bf16, tag="eb_sb")
                for j in range(8):
                    sl = slice(512 * j, 512 * (j + 1))
                    eb_ps = ppool.tile([128, 512], f32, tag="eb")
                    nc.tensor.matmul(eb_ps[:], lhsT=ones1[:], rhs=e_row[:, sl],
                                     start=True, stop=True)
                    nc.vector.tensor_copy(eb[:, sl], eb_ps[:])

                # ---- context sums: ctx[c] = sum_s x[c,s] * e[s]  (fused mul+acc)
                scr = fpool.tile([128, S], bf16, tag="scr")
                nc.vector.scalar_tensor_tensor(
                    out=scr[:], in0=xb0[:], scalar=1.0, in1=eb[:],
                    op0=ALU.mult, op1=ALU.mult, accum_out=stage[:, 2:3])
                nc.vector.scalar_tensor_tensor(
                    out=scr[:], in0=xb1[:], scalar=1.0, in1=eb[:],
                    op0=ALU.mult, op1=ALU.mult, accum_out=stage[:, 3:4])

                # ---- channel max: fold 256 -> 128 on gpsimd, then PE-transpose
                #      [128,128] chunks so the remaining 128-way max is a DVE
                #      free-axis reduce
                pm = wpool.tile([128, S], bf16, tag="pm")
                nc.vector.tensor_max(pm[:], xb0[:], xb1[:])
                rm = spool.tile([128, 32], bf16, tag="rm")
                for g in range(4):
                    ct_ps = ppool.tile([128, 1024], bf16, tag="ct")
                    for j in range(8):
                        cj = 8 * g + j
                        nc.tensor.transpose(ct_ps[:, 128 * j:128 * (j + 1)],
                                            pm[:, 128 * cj:128 * (cj + 1)],
                                            ident[:])
                    nc.vector.tensor_reduce(
                        rm[:, 8 * g:8 * (g + 1)],
                        ct_ps[:].rearrange("p (j c) -> p j c", c=128),
                        axis=AX.X, op=ALU.max)
                nc.vector.tensor_reduce(stage[:, 4:5], rm[:], axis=AX.X,
                                        op=ALU.add)

                nc.sync.dma_start(out=out_ext[b], in_=stage[:])
    return nc


def _get_nc():
    if "nc" not in _CACHE:
        nc = _build_nc()
        nc.finalize()
        _CACHE["nc"] = nc
    return _CACHE["nc"]


def _run_device(x_np, trace=False, tmpdir=None):
    """x_np: [64, 256, 64, 64] fp32 -> list of 8 per-core result dicts."""
    import ml_dtypes
    from concourse.bass_utils import run_bass_kernel_spmd

    nc = _get_nc()
    xs = x_np.reshape(NCORES, BPC, C, S)
    wm = _CACHE["w_mask"].reshape(C).astype(ml_dtypes.bfloat16)
    ones1 = np.ones([1, 128], dtype=ml_dtypes.bfloat16)
    ident = np.eye(128, dtype=ml_dtypes.bfloat16)
    in_maps = [
        {"x": np.ascontiguousarray(xs[i]), "wm": wm, "ones1": ones1, "ident": ident}
        for i in range(NCORES)
    ]
    res = run_bass_kernel_spmd(nc, in_maps, core_ids=list(range(NCORES)),
                               trace=trace, tmpdir=tmpdir)
    return res


def kernel(x, w_mask, b_mask, w_cm1, b_cm1, ln_w, ln_b, w_cm2, b_cm2,
           w_net1, w_net2, w_fc, bn_w, bn_b, bn_mean, bn_var, w_kfc):
    x = np.asarray(x, dtype=np.float32)
    _CACHE["w_mask"] = np.asarray(w_mask, dtype=np.float32)
    res = _run_device(x)

    # ---- gather device results
    beta_sums = np.zeros([B, C], np.float32)
    ctx_sums = np.zeros([B, C], np.float32)
    zs = np.zeros([B], np.float32)
    cmax_sums = np.zeros([B], np.float32)
    for i in range(NCORES):
        o = np.asarray(res.results[i]["out"], np.float32)  # [BPC, 128, 8]
        for bb in range(BPC):
            g = i * BPC + bb
            beta_sums[g, 0:128] = o[bb, :, 0]
            beta_sums[g, 128:256] = o[bb, :, 1]
            ctx_sums[g, 0:128] = o[bb, :, 2]
            ctx_sums[g, 128:256] = o[bb, :, 3]
            cmax_sums[g] = o[bb, :, 4].sum()
            zs[g] = o[bb, 0, 5] + o[bb, 0, 6]

    # ---- tiny epilogue head on host (mirrors reference.py)
    w_cm1 = np.asarray(w_cm1, np.float32); b_cm1 = np.asarray(b_cm1, np.float32)
    ln_w = np.asarray(ln_w, np.float32); ln_b = np.asarray(ln_b, np.float32)
    w_cm2 = np.asarray(w_cm2, np.float32); b_cm2 = np.asarray(b_cm2, np.float32)
    w_net1 = np.asarray(w_net1, np.float32); w_net2 = np.asarray(w_net2, np.float32)
    w_fc = np.asarray(w_fc, np.float32); bn_w = np.asarray(bn_w, np.float32)
    bn_b = np.asarray(bn_b, np.float32); bn_mean = np.asarray(bn_mean, np.float32)
    bn_var = np.asarray(bn_var, np.float32); w_kfc = np.asarray(w_kfc, np.float32)

    from scipy.special import erf  # exact gelu, matches jax approximate=False

    beta_c = beta_sums / S
    context = ctx_sums / zs[:, None]
    a = beta_sums.sum(axis=1) / (C * S)
    mm = cmax_sums / S
    beta_s = np.zeros([B, C], np.float32)
    beta_s[:, 0::2] = a[:, None]
    beta_s[:, 1::2] = mm[:, None]

    t = context @ w_cm1.T + b_cm1
    mu = t.mean(axis=-1, keepdims=True)
    var = ((t - mu) ** 2).mean(axis=-1, keepdims=True)
    t = (t - mu) / np.sqrt(var + EPS) * ln_w + ln_b
    t = t * 0.5 * (1.0 + erf(t / np.sqrt(2.0)))
    beta_g = t @ w_cm2.T + b_cm2

    out = beta_c + beta_g + beta_s
    out = np.maximum(out @ w_net1.T, 0.0) @ w_net2.T  # [B, K]

    ka = out @ w_fc.T
    ka = (ka - bn_mean) / np.sqrt(bn_var + EPS) * bn_w + bn_b
    kat = 1.0 / (1.0 + np.exp(-(np.maximum(ka, 0.0) @ w_kfc.T)))
    out = out * kat
    out = out / TEMP
    out = out - out.max(axis=-1, keepdims=True)
    e = np.exp(out)
    return (e / e.sum(axis=-1, keepdims=True)).astype(np.float32)


# revision 18
# speedup vs baseline: 498.7678x; 1.1796x over previous
"""Trainium2 Bass kernel for nn_Attention_63273458205325.

Data-parallel over batch: 64 images -> 8 NeuronCores x 8 images.
Device kernel computes, per image, the four memory-bound global
reductions over x[b] (256x4096 fp32):
  - beta row-sums  (per-channel sum over spatial)       [256]
  - mask logits m = w_mask . x  -> exp -> Z and the
    softmax-weighted context sums  sum_s x[c,s]*e[s]    [256]
  - mean over spatial of (max over channels)            scalar
The tiny [B,8] epilogue head (layernorm/gelu/1x1 convs/sigmoid/softmax
on 256-vectors) runs on host.
"""

import sys

import numpy as np

sys.path.insert(0, "/opt/trn_rl_repo")

B, C, H, W = 64, 256, 64, 64
S = H * W  # 4096
NCORES = 8
BPC = B // NCORES  # images per core
RATIO, K = 16, 8
PLANES = C // 2
HIDDEN = C // RATIO
TEMP = 30.0
EPS = 1e-5

_CACHE = {}


def _build_nc():
    import concourse.bacc as bacc
    import concourse.mybir as mybir
    from concourse.tile import TileContext

    f32 = mybir.dt.float32
    bf16 = mybir.dt.bfloat16
    AF = mybir.ActivationFunctionType
    ALU = mybir.AluOpType
    AX = mybir.AxisListType

    nc = bacc.Bacc(None, target_bir_lowering=False)

    x_ext = nc.declare_dram_parameter("x", [BPC, C, S], f32, isOutput=False)
    wm_ext = nc.declare_dram_parameter("wm", [C], bf16, isOutput=False)
    ones_ext = nc.declare_dram_parameter("ones1", [1, 128], bf16, isOutput=False)
    id_ext = nc.declare_dram_parameter("ident", [128, 128], bf16, isOutput=False)
    out_ext = nc.declare_dram_parameter("out", [BPC, 128, 8], f32, isOutput=True)

    with TileContext(nc) as tc:
        with (
            tc.tile_pool(name="const", bufs=1) as cpool,
            tc.tile_pool(name="xin", bufs=2) as xpool,
            tc.tile_pool(name="work", bufs=2) as wpool,
            tc.tile_pool(name="fold", bufs=1) as fpool,
            tc.tile_pool(name="small", bufs=2) as spool,
            tc.tile_pool(name="psum", bufs=2, space="PSUM") as ppool,
        ):
            # constants
            wm = cpool.tile([128, 2], bf16)  # wm[p, g] = w_mask[g*128 + p]
            nc.sync.dma_start(out=wm[:], in_=wm_ext.rearrange("(g p) -> p g", p=128))
            ones1 = cpool.tile([1, 128], bf16)
            nc.sync.dma_start(out=ones1[:], in_=ones_ext[:])
            ident = cpool.tile([128, 128], bf16)
            nc.sync.dma_start(out=ident[:], in_=id_ext[:])

            for b in range(BPC):
                # ---- load both channel halves [128, 4096] fp32
                x0 = xpool.tile([128, S], f32, tag="x0")
                nc.sync.dma_start(out=x0[:], in_=x_ext[b, 0:128, :])
                x1 = xpool.tile([128, S], f32, tag="x1")
                nc.sync.dma_start(out=x1[:], in_=x_ext[b, 128:256, :])

                stage = spool.tile([128, 8], f32, tag="stage")
                nc.gpsimd.memset(stage[:], 0.0)

                # ---- cast to bf16 on ACT; channel row-sums ride along for free
                xb0 = wpool.tile([128, S], bf16, tag="xb0")
                nc.scalar.activation(xb0[:], x0[:], AF.Copy,
                                     accum_out=stage[:, 0:1])
                xb1 = wpool.tile([128, S], bf16, tag="xb1")
                nc.scalar.activation(xb1[:], x1[:], AF.Copy,
                                     accum_out=stage[:, 1:2])

                # ---- mask logits m = w . x  -> PSUM, four [1, 1024] chunks
                #      e = exp(m) (bf16 row); Z = sum(e) split into 4 accums
                e_row = spool.tile([1, S], bf16, tag="e")
                zacc = spool.tile([1, 4], f32, tag="z")
                for h in range(4):
                    m_ps = ppool.tile([1, S // 4], f32, tag="m")
                    for j in range(2):
                        sl = slice(512 * j, 512 * (j + 1))
                        gl = slice(1024 * h + 512 * j, 1024 * h + 512 * (j + 1))
                        nc.tensor.matmul(m_ps[:, sl], lhsT=wm[:, 0:1],
                                         rhs=xb0[:, gl], start=True, stop=False)
                        nc.tensor.matmul(m_ps[:, sl], lhsT=wm[:, 1:2],
                                         rhs=xb1[:, gl], start=False, stop=True)
                    nc.scalar.activation(e_row[:, 1024 * h:1024 * (h + 1)],
                                         m_ps[:], AF.Exp,
                                         accum_out=zacc[:, h:h + 1])
                nc.vector.tensor_reduce(stage[0:1, 5:6], zacc[:], axis=AX.X,
                                        op=ALU.add)

                # ---- broadcast e via K=1 matmul; ctx partials read PSUM direct
                cacc = spool.tile([128, 16], f32, tag="cacc")
                scr = fpool.tile([128, 512], bf16, tag="scr")
                for j in range(8):
                    sl = slice(512 * j, 512 * (j + 1))
                    eb_ps = ppool.tile([128, 512], f32, tag="eb")
                    nc.tensor.matmul(eb_ps[:], lhsT=ones1[:], rhs=e_row[:, sl],
                                     start=True, stop=True)
                    nc.vector.scalar_tensor_tensor(
                        out=scr[:], in0=xb0[:, sl], scalar=1.0, in1=eb_ps[:],
                        op0=ALU.mult, op1=ALU.mult, accum_out=cacc[:, j:j + 1])
                    nc.vector.scalar_tensor_tensor(
                        out=scr[:], in0=xb1[:, sl], scalar=1.0, in1=eb_ps[:],
                        op0=ALU.mult, op1=ALU.mult, accum_out=cacc[:, 8 + j:9 + j])
                nc.vector.tensor_reduce(
                    stage[:, 2:4], cacc[:].rearrange("p (c j) -> p c j", j=8),
                    axis=AX.X, op=ALU.add)

                # ---- channel max: fold 256 -> 128 on gpsimd, then PE-transpose
                #      [128,128] chunks so the remaining 128-way max is a DVE
                #      free-axis reduce
                pm = wpool.tile([128, S], bf16, tag="pm")
                nc.vector.tensor_max(pm[:], xb0[:], xb1[:])
                rm = spool.tile([128, 32], bf16, tag="rm")
                for g in range(4):
                    ct_ps = ppool.tile([128, 1024], bf16, tag="ct")
                    for j in range(8):
                        cj = 8 * g + j
                        nc.tensor.transpose(ct_ps[:, 128 * j:128 * (j + 1)],
                                            pm[:, 128 * cj:128 * (cj + 1)],
                                            ident[:])
                    nc.vector.tensor_reduce(
                        rm[:, 8 * g:8 * (g + 1)],
                        ct_ps[:].rearrange("p (j c) -> p j c", c=128),
                        axis=AX.X, op=ALU.max)
                nc.vector.tensor_reduce(stage[:, 4:5], rm[:], axis=AX.X,
                                        op=ALU.add)

                nc.sync.dma_start(out=out_ext[b], in_=stage[:])
    return nc


def _get_nc():
    if "nc" not in _CACHE:
        nc = _build_nc()
        nc.finalize()
        _CACHE["nc"] = nc
    return _CACHE["nc"]


def _run_device(x_np, trace=False, tmpdir=None):
    """x_np: [64, 256, 64, 64] fp32 -> list of 8 per-core result dicts."""
    import ml_dtypes
    from concourse.bass_utils import run_bass_kernel_spmd

    nc = _get_nc()
    xs = x_np.reshape(NCORES, BPC, C, S)
    wm = _CACHE["w_mask"].reshape(C).astype(ml_dtypes.bfloat16)
    ones1 = np.ones([1, 128], dtype=ml_dtypes.bfloat16)
    ident = np.eye(128, dtype=ml_dtypes.bfloat16)
    in_maps = [
        {"x": np.ascontiguousarray(xs[i]), "wm": wm, "ones1": ones1, "ident": ident}
        for i in range(NCORES)
    ]
    res = run_bass_kernel_spmd(nc, in_maps, core_ids=list(range(NCORES)),
                               trace=trace, tmpdir=tmpdir)
    return res


def kernel(x, w_mask, b_mask, w_cm1, b_cm1, ln_w, ln_b, w_cm2, b_cm2,
           w_net1, w_net2, w_fc, bn_w, bn_b, bn_mean, bn_var, w_kfc):
    x = np.asarray(x, dtype=np.float32)
    _CACHE["w_mask"] = np.asarray(w_mask, dtype=np.float32)
    res = _run_device(x)

    # ---- gather device results
    beta_sums = np.zeros([B, C], np.float32)
    ctx_sums = np.zeros([B, C], np.float32)
    zs = np.zeros([B], np.float32)
    cmax_sums = np.zeros([B], np.float32)
    for i in range(NCORES):
        o = np.asarray(res.results[i]["out"], np.float32)  # [BPC, 128, 8]
        for bb in range(BPC):
            g = i * BPC + bb
            beta_sums[g, 0:128] = o[bb, :, 0]
            beta_sums[g, 128:256] = o[bb, :, 1]
            ctx_sums[g, 0:128] = o[bb, :, 2]
            ctx_sums[g, 128:256] = o[bb, :, 3]
            cmax_sums[g] = o[bb, :, 4].sum()
            zs[g] = o[bb, 0, 5]

    # ---- tiny epilogue head on host (mirrors reference.py)
    w_cm1 = np.asarray(w_cm1, np.float32); b_cm1 = np.asarray(b_cm1, np.float32)
    ln_w = np.asarray(ln_w, np.float32); ln_b = np.asarray(ln_b, np.float32)
    w_cm2 = np.asarray(w_cm2, np.float32); b_cm2 = np.asarray(b_cm2, np.float32)
    w_net1 = np.asarray(w_net1, np.float32); w_net2 = np.asarray(w_net2, np.float32)
    w_fc = np.asarray(w_fc, np.float32); bn_w = np.asarray(bn_w, np.float32)
    bn_b = np.asarray(bn_b, np.float32); bn_mean = np.asarray(bn_mean, np.float32)
    bn_var = np.asarray(bn_var, np.float32); w_kfc = np.asarray(w_kfc, np.float32)

    from scipy.special import erf  # exact gelu, matches jax approximate=False

    beta_c = beta_sums / S
    context = ctx_sums / zs[:, None]
    a = beta_sums.sum(axis=1) / (C * S)
    mm = cmax_sums / S
    beta_s = np.zeros([B, C], np.float32)
    beta_s[:, 0::2] = a[:, None]
    beta_s[:, 1::2] = mm[:, None]

    t = context @ w_cm1.T + b_cm1
    mu = t.mean(axis=-1, keepdims=True)
    var = ((t - mu) ** 2).mean(axis=-1, keepdims=True)
    t = (t - mu) / np.sqrt(var + EPS) * ln_w + ln_b
    t = t * 0.5 * (1.0 + erf(t / np.sqrt(2.0)))
    beta_g = t @ w_cm2.T + b_cm2

    out = beta_c + beta_g + beta_s
    out = np.maximum(out @ w_net1.T, 0.0) @ w_net2.T  # [B, K]

    ka = out @ w_fc.T
    ka = (ka - bn_mean) / np.sqrt(bn_var + EPS) * bn_w + bn_b
    kat = 1.0 / (1.0 + np.exp(-(np.maximum(ka, 0.0) @ w_kfc.T)))
    out = out * kat
    out = out / TEMP
    out = out - out.max(axis=-1, keepdims=True)
    e = np.exp(out)
    return (e / e.sum(axis=-1, keepdims=True)).astype(np.float32)
